# revision 28
# baseline (speedup 1.0000x reference)
"""CoefficientMaxPool Trainium2 kernel (8-core data-parallel).

Problem: x [32, 512, 16, 128] f32.  Irreps group into degree blocks
l=0:[0,1), l=1:[1,4), l=2:[4,9), l=3:[9,16).  Per (batch, l, channel):
find the neighbor n* maximizing the degree-block squared norm, output
that neighbor's block components -> out [32, 16, 128].

Per core (4 batches), per batch:
  - DMA x[b] as [p=128, a=4, i=16, c=128] (n = a*128 + p), per-a chunks
  - ACT: x2 = x*x
  - PE transpose-accumulate x2 i-planes -> NT_l [c, n] in PSUM (fp32,
    exact argmax), processed per degree l (l3 first) so only 2 NT
    banks are live and the longest downstream chain starts earliest
  - DVE: maxv_l[c] = reduce-max over n; maskT_l[c,n] = (NT_l == maxv_l)
    emitted as bf16 (0/1 exact, so no precision risk in the select)
  - PE: transpose bf16 masks back -> MP_l [n, (a c)] PSUM (1 cyc/row)
  - winner-select Xb = X * MP[l(i)] -> bf16: l3 on GPSIMD (via an
    ACT-bounced SBUF mask copy; GPSIMD cannot read PSUM), l2+l1+l0 on
    DVE reading PSUM masks directly.  DVE's EQs all flow before its
    winner-mults so the PE's mask transposes are never starved.
  - PE: ones^T @ Xb (bf16: 1 cyc/row) accumulated over a -> out halves
    [1, 2, 512] fp32 in PSUM; ACT copy -> SBUF; ACT-issued DMA out.

Scheduling notes (each worth real microseconds on hardware):
  - The masked sum for batch b is emitted at the END of iteration b+1's
    PE program, so the PE fills batch b's winner-select wait with batch
    b+1's transposes.
  - The PSUM->SBUF out copy runs two iterations late: by then its PSUM
    source is guaranteed ready, so it never blocks ACT's squares.
  - Out DMAs are issued from the ACT queue: on the sync queue they
    would head-of-line block the next batch's input DMA.
  - X tiles are triple-buffered: the input DMA for batch b+3 only
    waits on batch b's last winner-mult, keeping the input stream at
    full HBM bandwidth (the single input queue is the ~47us floor).
PSUM banks: 2 (NT ring) + 2 (mask ring) + 4 (out halves ring) = 8.
"""

import os
import sys

import numpy as np

for _p in ("/opt/trn_rl_repo", "/opt/pypackages"):
    if _p not in sys.path:
        sys.path.append(_p)

from contextlib import ExitStack

import concourse.bacc as bacc
import concourse.bass as bass
import concourse.tile as tile
from concourse import mybir

N_CORES = 8
B_FULL, N, IRR, C = 32, 512, 16, 128
B = B_FULL // N_CORES  # 4 batches per core
P = 128                # partitions (n within chunk)
A = N // P             # 4 neighbor chunks
BLOCKS = [(0, 1), (1, 4), (4, 9), (9, 16)]  # irrep ranges per degree l
F32 = mybir.dt.float32
BF16 = mybir.dt.bfloat16
MAX = mybir.AluOpType.max
MULT = mybir.AluOpType.mult
EQ = mybir.AluOpType.is_equal

_cache = {}


def _build_bass():
    nc = bacc.Bacc("TRN2", target_bir_lowering=False, debug=False,
                   num_devices=N_CORES)
    x_in = nc.dram_tensor("x", [B, N, IRR, C], F32, kind="ExternalInput")
    out_t = nc.dram_tensor("out", [B, IRR, C], F32, kind="ExternalOutput")
    ident_d = nc.inline_tensor(np.eye(P, dtype=np.float32), name="ident")

    with tile.TileContext(nc) as tc, ExitStack() as ctx:
        # DRAM view: n = a*P + p  ->  [b, p, a, i, c]
        x_v = x_in.ap().rearrange("b (a p) i c -> b p a i c", p=P)
        out_v = out_t.ap().rearrange("b i c -> b (i c)")

        xp = ctx.enter_context(tc.tile_pool(name="xp", bufs=3))
        x2p = ctx.enter_context(tc.tile_pool(name="x2p", bufs=2))
        xbp = ctx.enter_context(tc.tile_pool(name="xbp", bufs=2))
        mtp = ctx.enter_context(tc.tile_pool(name="mtp", bufs=1))
        msp = ctx.enter_context(tc.tile_pool(name="msp", bufs=2))
        mvp = ctx.enter_context(tc.tile_pool(name="mvp", bufs=2))
        obp = ctx.enter_context(tc.tile_pool(name="obp", bufs=1))
        singles = ctx.enter_context(tc.tile_pool(name="singles", bufs=1))
        # PSUM banks: pnt 2 + pmp 2 + pout 2x2 = 8
        pnt = ctx.enter_context(tc.tile_pool(name="pnt", bufs=2, space="PSUM"))
        pmp = ctx.enter_context(tc.tile_pool(name="pmp", bufs=2, space="PSUM"))
        pout = ctx.enter_context(tc.tile_pool(name="pout", bufs=2, space="PSUM"))

        ones_b = singles.tile([P, 1], BF16)
        nc.vector.memset(ones_b, 1.0)
        ident = singles.tile([P, P], F32)
        nc.sync.dma_start(out=ident, in_=ident_d.ap())
        ident_b = singles.tile([P, P], BF16)
        nc.scalar.copy(out=ident_b, in_=ident)

        Xbs, POs = {}, {}

        def transpose_norms(l, X2):
            NT = pnt.tile([P, A * P], F32, tag="nt", name=f"NT{l}")
            s, e = BLOCKS[l]
            for a in range(A):
                sl = slice(a * P, (a + 1) * P)
                for i in range(s, e):
                    nc.tensor.matmul(NT[:, sl], X2[:, a, i, :], ident,
                                     is_transpose=True,
                                     start=(i == s), stop=(i == e - 1))
            return NT

        def mask_of(l, NT):
            maxv = mvp.tile([P, 1], F32, tag=f"maxv{l}", name=f"maxv{l}")
            nc.vector.tensor_reduce(
                out=maxv, in_=NT, axis=mybir.AxisListType.X, op=MAX)
            mt = mtp.tile([P, A * P], BF16, tag=f"mt{l}", name=f"mt{l}")
            nc.vector.tensor_scalar(
                out=mt, in0=NT, scalar1=maxv, scalar2=None, op0=EQ)
            mp = pmp.tile([P, A * P], BF16, tag="mp", name=f"mp{l}")
            for a in range(A):
                sl = slice(a * P, (a + 1) * P)
                nc.tensor.matmul(mp[:, sl], mt[:, sl], ident_b,
                                 is_transpose=True)
            return mp

        for it in range(B + 2):
            if it < B:
                b = it
                X = xp.tile([P, A, IRR, C], F32, tag="X", name="X")
                X2 = x2p.tile([P, A, IRR, C], F32, tag="X2", name="X2")
                Xb = xbp.tile([P, A, IRR, C], BF16, tag="Xb", name="Xb")
                Xbs[b] = Xb
                for a in range(A):
                    nc.sync.dma_start(out=X[:, a], in_=x_v[b][:, a])
                if b == 0:
                    # per-chunk squares so the first transposes start early
                    # (batch 0 is input-arrival-limited)
                    for a in range(A):
                        nc.scalar.activation(X2[:, a], X[:, a],
                                             mybir.ActivationFunctionType.Square)
                else:
                    for h in range(2):
                        ha = slice(2 * h, 2 * h + 2)
                        nc.scalar.activation(X2[:, ha], X[:, ha],
                                             mybir.ActivationFunctionType.Square)

                def wm_dve(l, mp):
                    s, e = BLOCKS[l]
                    mask = mp.rearrange("p (a c) -> p a c", a=A)
                    nc.vector.tensor_tensor(
                        Xb[:, :, s:e, :], X[:, :, s:e, :],
                        mask.unsqueeze(2).broadcast_to([P, A, e - s, C]),
                        MULT)

                def mask_sbuf(l, mp):
                    # GPSIMD cannot read PSUM: bounce the mask via ACT
                    ms = msp.tile([P, A, C], BF16, tag=f"ms{l}",
                                  name=f"ms{l}")
                    nc.scalar.copy(out=ms,
                                   in_=mp.rearrange("p (a c) -> p a c", a=A))
                    return ms

                def wm_gps(l, ms):
                    s, e = BLOCKS[l]
                    nc.gpsimd.tensor_tensor(
                        Xb[:, :, s:e, :], X[:, :, s:e, :],
                        ms.unsqueeze(2).broadcast_to([P, A, e - s, C]),
                        MULT)

                def emit_msum(bm):
                    # masked sum over n for batch bm, one iteration late so
                    # the PE never stalls on bm's winner-select
                    Xf = Xbs[bm].rearrange("p a i c -> p a (i c)")
                    POs[bm] = []
                    for h in range(2):
                        po = pout.tile([1, 2, 512], F32, tag="po", name="po")
                        for kk in range(2):
                            k = 2 * h + kk
                            for a in range(A):
                                nc.tensor.matmul(
                                    po[:, kk, :], ones_b,
                                    Xf[:, a, k * 512:(k + 1) * 512],
                                    start=(a == 0), stop=(a == A - 1))
                        POs[bm].append(po)

                # l3 FIRST: the GPSIMD winner-mult (7us, the longest X
                # reader) gets its mask as early as possible. Each l's
                # max/EQ runs under the next l's norm transposes, and DVE's
                # EQs all flow BEFORE its winner-mults (a winner-mult in
                # between would stall the PE's next mask transpose). The
                # previous batch's masked sum slots into the PE program
                # mid-iteration, after mask-l2's transpose.
                NT3 = transpose_norms(3, X2)
                NT2 = transpose_norms(2, X2)
                MP3 = mask_of(3, NT3)
                NT1 = transpose_norms(1, X2)
                wm_gps(3, mask_sbuf(3, MP3))
                MP2 = mask_of(2, NT2)
                NT0 = transpose_norms(0, X2)
                MP1 = mask_of(1, NT1)
                wm_dve(2, MP2)
                MP0 = mask_of(0, NT0)
                wm_dve(1, MP1)
                wm_dve(0, MP0)

            if it >= 2:
                # out copy + store for batch it-2, per k-chunk in completion
                # order so the out DMA fires as soon as possible
                bo = it - 2
                ob = obp.tile([1, IRR * C], F32, tag="ob", name="ob")
                for h in range(2):
                    nc.scalar.copy(out=ob[:, h * 1024:(h + 1) * 1024],
                                   in_=POs[bo][h].rearrange("m k f -> m (k f)"))
                # issue from ACT: keeps the sync queue a pure input stream
                # (an out DMA there would head-of-line block the next
                # batch's input DMA behind the out copy)
                nc.scalar.dma_start(out=out_v[bo].unsqueeze(0), in_=ob)

            if 1 <= it <= B:
                emit_msum(it - 1)

    nc.compile()
    return nc


def kernel(x: np.ndarray, i2l: np.ndarray | None = None) -> np.ndarray:
    x = np.ascontiguousarray(np.asarray(x), dtype=np.float32)
    assert x.shape == (B_FULL, N, IRR, C), x.shape

    if "nc" not in _cache:
        _cache["nc"] = _build_bass()
    nc = _cache["nc"]

    from concourse.bass_utils import run_bass_kernel_spmd

    in_maps = [{"x": x[i * B:(i + 1) * B]} for i in range(N_CORES)]
    res = run_bass_kernel_spmd(nc, in_maps, list(range(N_CORES)))
    out = np.concatenate([res.results[i]["out"] for i in range(N_CORES)], axis=0)
    return out


if __name__ == "__main__":
    xs = np.random.randn(B_FULL, N, IRR, C).astype(np.float32)
    o = kernel(xs)
    print("out", o.shape, o.dtype)


# revision 29
# speedup vs baseline: 1.0009x; 1.0009x over previous
"""CoefficientMaxPool Trainium2 kernel (8-core data-parallel).

Problem: x [32, 512, 16, 128] f32.  Irreps group into degree blocks
l=0:[0,1), l=1:[1,4), l=2:[4,9), l=3:[9,16).  Per (batch, l, channel):
find the neighbor n* maximizing the degree-block squared norm, output
that neighbor's block components -> out [32, 16, 128].

Per core (4 batches), per batch:
  - DMA x[b] as [p=128, a=4, i=16, c=128] (n = a*128 + p), per-a chunks
  - ACT: x2 = x*x
  - PE transpose-accumulate x2 i-planes -> NT_l [c, n] in PSUM (fp32,
    exact argmax), processed per degree l (l3 first) so only 2 NT
    banks are live and the longest downstream chain starts earliest
  - DVE: maxv_l[c] = reduce-max over n; maskT_l[c,n] = (NT_l == maxv_l)
    emitted as bf16 (0/1 exact, so no precision risk in the select)
  - PE: transpose bf16 masks back -> MP_l [n, (a c)] PSUM (1 cyc/row)
  - winner-select Xb = X * MP[l(i)] -> bf16: l3 on GPSIMD (via an
    ACT-bounced SBUF mask copy; GPSIMD cannot read PSUM), l2+l1+l0 on
    DVE reading PSUM masks directly.  DVE's EQs all flow before its
    winner-mults so the PE's mask transposes are never starved.
  - PE: ones^T @ Xb (bf16: 1 cyc/row) accumulated over a -> out halves
    [1, 2, 512] fp32 in PSUM; ACT copy -> SBUF; ACT-issued DMA out.

Scheduling notes (each worth real microseconds on hardware):
  - The masked sum for batch b is emitted at the END of iteration b+1's
    PE program, so the PE fills batch b's winner-select wait with batch
    b+1's transposes.
  - The PSUM->SBUF out copy runs two iterations late: by then its PSUM
    source is guaranteed ready, so it never blocks ACT's squares.
  - Out DMAs are issued from the ACT queue: on the sync queue they
    would head-of-line block the next batch's input DMA.
  - X tiles are triple-buffered: the input DMA for batch b+3 only
    waits on batch b's last winner-mult, keeping the input stream at
    full HBM bandwidth (the single input queue is the ~47us floor).
PSUM banks: 2 (NT ring) + 2 (mask ring) + 4 (out halves ring) = 8.
"""

import os
import sys

import numpy as np

for _p in ("/opt/trn_rl_repo", "/opt/pypackages"):
    if _p not in sys.path:
        sys.path.append(_p)

from contextlib import ExitStack

import concourse.bacc as bacc
import concourse.bass as bass
import concourse.tile as tile
from concourse import mybir

N_CORES = 8
B_FULL, N, IRR, C = 32, 512, 16, 128
B = B_FULL // N_CORES  # 4 batches per core
P = 128                # partitions (n within chunk)
A = N // P             # 4 neighbor chunks
BLOCKS = [(0, 1), (1, 4), (4, 9), (9, 16)]  # irrep ranges per degree l
F32 = mybir.dt.float32
BF16 = mybir.dt.bfloat16
MAX = mybir.AluOpType.max
MULT = mybir.AluOpType.mult
EQ = mybir.AluOpType.is_equal

_cache = {}


def _build_bass():
    nc = bacc.Bacc("TRN2", target_bir_lowering=False, debug=False,
                   num_devices=N_CORES)
    x_in = nc.dram_tensor("x", [B, N, IRR, C], F32, kind="ExternalInput")
    out_t = nc.dram_tensor("out", [B, IRR, C], F32, kind="ExternalOutput")
    ident_d = nc.inline_tensor(np.eye(P, dtype=np.float32), name="ident")

    with tile.TileContext(nc) as tc, ExitStack() as ctx:
        # DRAM view: n = a*P + p  ->  [b, p, a, i, c]
        x_v = x_in.ap().rearrange("b (a p) i c -> b p a i c", p=P)
        out_v = out_t.ap().rearrange("b i c -> b (i c)")

        xp = ctx.enter_context(tc.tile_pool(name="xp", bufs=3))
        x2p = ctx.enter_context(tc.tile_pool(name="x2p", bufs=2))
        xbp = ctx.enter_context(tc.tile_pool(name="xbp", bufs=2))
        mtp = ctx.enter_context(tc.tile_pool(name="mtp", bufs=1))
        msp = ctx.enter_context(tc.tile_pool(name="msp", bufs=2))
        mvp = ctx.enter_context(tc.tile_pool(name="mvp", bufs=2))
        obp = ctx.enter_context(tc.tile_pool(name="obp", bufs=1))
        singles = ctx.enter_context(tc.tile_pool(name="singles", bufs=1))
        # PSUM banks: pnt 2 + pmp 2 + pout 2x2 = 8
        pnt = ctx.enter_context(tc.tile_pool(name="pnt", bufs=2, space="PSUM"))
        pmp = ctx.enter_context(tc.tile_pool(name="pmp", bufs=2, space="PSUM"))
        pout = ctx.enter_context(tc.tile_pool(name="pout", bufs=2, space="PSUM"))

        ones_b = singles.tile([P, 1], BF16)
        nc.vector.memset(ones_b, 1.0)
        ident = singles.tile([P, P], F32)
        nc.sync.dma_start(out=ident, in_=ident_d.ap())
        ident_b = singles.tile([P, P], BF16)
        nc.scalar.copy(out=ident_b, in_=ident)

        Xbs, POs = {}, {}

        def transpose_norms(l, X2):
            NT = pnt.tile([P, A * P], F32, tag="nt", name=f"NT{l}")
            s, e = BLOCKS[l]
            for a in range(A):
                sl = slice(a * P, (a + 1) * P)
                for i in range(s, e):
                    nc.tensor.matmul(NT[:, sl], X2[:, a, i, :], ident,
                                     is_transpose=True,
                                     start=(i == s), stop=(i == e - 1))
            return NT

        def mask_of(l, NT):
            maxv = mvp.tile([P, 1], F32, tag=f"maxv{l}", name=f"maxv{l}")
            nc.vector.tensor_reduce(
                out=maxv, in_=NT, axis=mybir.AxisListType.X, op=MAX)
            mt = mtp.tile([P, A * P], BF16, tag=f"mt{l}", name=f"mt{l}")
            nc.vector.tensor_scalar(
                out=mt, in0=NT, scalar1=maxv, scalar2=None, op0=EQ)
            mp = pmp.tile([P, A * P], BF16, tag="mp", name=f"mp{l}")
            for a in range(A):
                sl = slice(a * P, (a + 1) * P)
                nc.tensor.matmul(mp[:, sl], mt[:, sl], ident_b,
                                 is_transpose=True)
            return mp

        for it in range(B + 2):
            if it < B:
                b = it
                X = xp.tile([P, A, IRR, C], F32, tag="X", name="X")
                X2 = x2p.tile([P, A, IRR, C], F32, tag="X2", name="X2")
                Xb = xbp.tile([P, A, IRR, C], BF16, tag="Xb", name="Xb")
                Xbs[b] = Xb
                for a in range(A):
                    nc.sync.dma_start(out=X[:, a], in_=x_v[b][:, a])
                # per-chunk squares: each chunk's transposes start as soon
                # as that chunk is squared, shortening the chain to the
                # masks (worth the extra ACT instruction overhead)
                for a in range(A):
                    nc.scalar.activation(X2[:, a], X[:, a],
                                         mybir.ActivationFunctionType.Square)

                def wm_dve(l, mp):
                    s, e = BLOCKS[l]
                    mask = mp.rearrange("p (a c) -> p a c", a=A)
                    nc.vector.tensor_tensor(
                        Xb[:, :, s:e, :], X[:, :, s:e, :],
                        mask.unsqueeze(2).broadcast_to([P, A, e - s, C]),
                        MULT)

                def mask_sbuf(l, mp):
                    # GPSIMD cannot read PSUM: bounce the mask via ACT
                    ms = msp.tile([P, A, C], BF16, tag=f"ms{l}",
                                  name=f"ms{l}")
                    nc.scalar.copy(out=ms,
                                   in_=mp.rearrange("p (a c) -> p a c", a=A))
                    return ms

                def wm_gps(l, ms):
                    s, e = BLOCKS[l]
                    nc.gpsimd.tensor_tensor(
                        Xb[:, :, s:e, :], X[:, :, s:e, :],
                        ms.unsqueeze(2).broadcast_to([P, A, e - s, C]),
                        MULT)

                def emit_msum(bm):
                    # masked sum over n for batch bm, one iteration late so
                    # the PE never stalls on bm's winner-select
                    Xf = Xbs[bm].rearrange("p a i c -> p a (i c)")
                    POs[bm] = []
                    for h in range(2):
                        po = pout.tile([1, 2, 512], F32, tag="po", name="po")
                        for kk in range(2):
                            k = 2 * h + kk
                            for a in range(A):
                                nc.tensor.matmul(
                                    po[:, kk, :], ones_b,
                                    Xf[:, a, k * 512:(k + 1) * 512],
                                    start=(a == 0), stop=(a == A - 1))
                        POs[bm].append(po)

                # l3 FIRST: the GPSIMD winner-mult (7us, the longest X
                # reader) gets its mask as early as possible. Each l's
                # max/EQ runs under the next l's norm transposes, and DVE's
                # EQs all flow BEFORE its winner-mults (a winner-mult in
                # between would stall the PE's next mask transpose). The
                # previous batch's masked sum slots into the PE program
                # mid-iteration, after mask-l2's transpose.
                NT3 = transpose_norms(3, X2)
                NT2 = transpose_norms(2, X2)
                MP3 = mask_of(3, NT3)
                NT1 = transpose_norms(1, X2)
                wm_gps(3, mask_sbuf(3, MP3))
                MP2 = mask_of(2, NT2)
                NT0 = transpose_norms(0, X2)
                MP1 = mask_of(1, NT1)
                wm_dve(2, MP2)
                MP0 = mask_of(0, NT0)
                wm_dve(1, MP1)
                wm_dve(0, MP0)

            if it >= 2:
                # out copy + store for batch it-2, per k-chunk in completion
                # order so the out DMA fires as soon as possible
                bo = it - 2
                ob = obp.tile([1, IRR * C], F32, tag="ob", name="ob")
                for h in range(2):
                    nc.scalar.copy(out=ob[:, h * 1024:(h + 1) * 1024],
                                   in_=POs[bo][h].rearrange("m k f -> m (k f)"))
                # issue from ACT: keeps the sync queue a pure input stream
                # (an out DMA there would head-of-line block the next
                # batch's input DMA behind the out copy)
                nc.scalar.dma_start(out=out_v[bo].unsqueeze(0), in_=ob)

            if 1 <= it <= B:
                emit_msum(it - 1)

    nc.compile()
    return nc


def kernel(x: np.ndarray, i2l: np.ndarray | None = None) -> np.ndarray:
    x = np.ascontiguousarray(np.asarray(x), dtype=np.float32)
    assert x.shape == (B_FULL, N, IRR, C), x.shape

    if "nc" not in _cache:
        _cache["nc"] = _build_bass()
    nc = _cache["nc"]

    from concourse.bass_utils import run_bass_kernel_spmd

    in_maps = [{"x": x[i * B:(i + 1) * B]} for i in range(N_CORES)]
    res = run_bass_kernel_spmd(nc, in_maps, list(range(N_CORES)))
    out = np.concatenate([res.results[i]["out"] for i in range(N_CORES)], axis=0)
    return out


if __name__ == "__main__":
    xs = np.random.randn(B_FULL, N, IRR, C).astype(np.float32)
    o = kernel(xs)
    print("out", o.shape, o.dtype)


# revision 31
# speedup vs baseline: 1.0334x; 1.0324x over previous
"""CoefficientMaxPool Trainium2 kernel (8-core data-parallel).

Problem: x [32, 512, 16, 128] f32.  Irreps group into degree blocks
l=0:[0,1), l=1:[1,4), l=2:[4,9), l=3:[9,16).  Per (batch, l, channel):
find the neighbor n* maximizing the degree-block squared norm, output
that neighbor's block components -> out [32, 16, 128].

Per core (4 batches), per batch:
  - DMA x[b] as [p=128, a=4, i=16, c=128] (n = a*128 + p), per-a chunks
  - ACT: x2 = x*x
  - PE transpose-accumulate x2 i-planes -> NT_l [c, n] in PSUM (fp32,
    exact argmax), processed per degree l (l3 first) so only 2 NT
    banks are live and the longest downstream chain starts earliest
  - DVE: maxv_l[c] = reduce-max over n; maskT_l[c,n] = (NT_l == maxv_l)
    emitted as bf16 (0/1 exact, so no precision risk in the select)
  - PE: transpose bf16 masks back -> MP_l [n, (a c)] PSUM (1 cyc/row)
  - winner-select Xb = X * MP[l(i)] -> bf16: l3 on GPSIMD (via an
    ACT-bounced SBUF mask copy; GPSIMD cannot read PSUM), l2+l1+l0 on
    DVE reading PSUM masks directly.  DVE's EQs all flow before its
    winner-mults so the PE's mask transposes are never starved.
  - PE: ones^T @ Xb (bf16: 1 cyc/row) accumulated over a -> out halves
    [1, 2, 512] fp32 in PSUM; ACT copy -> SBUF; ACT-issued DMA out.

Scheduling notes (each worth real microseconds on hardware):
  - The masked sum for batch b is emitted at the END of iteration b+1's
    PE program, so the PE fills batch b's winner-select wait with batch
    b+1's transposes.
  - The PSUM->SBUF out copy runs two iterations late: by then its PSUM
    source is guaranteed ready, so it never blocks ACT's squares.
  - Out DMAs are issued from the ACT queue: on the sync queue they
    would head-of-line block the next batch's input DMA.
  - X tiles are triple-buffered: the input DMA for batch b+3 only
    waits on batch b's last winner-mult, keeping the input stream at
    full HBM bandwidth (the single input queue is the ~47us floor).
PSUM banks: 2 (NT ring) + 2 (mask ring) + 4 (out halves ring) = 8.
"""

import os
import sys

import numpy as np

for _p in ("/opt/trn_rl_repo", "/opt/pypackages"):
    if _p not in sys.path:
        sys.path.append(_p)

from contextlib import ExitStack

import concourse.bacc as bacc
import concourse.bass as bass
import concourse.tile as tile
from concourse import mybir

N_CORES = 8
B_FULL, N, IRR, C = 32, 512, 16, 128
B = B_FULL // N_CORES  # 4 batches per core
P = 128                # partitions (n within chunk)
A = N // P             # 4 neighbor chunks
BLOCKS = [(0, 1), (1, 4), (4, 9), (9, 16)]  # irrep ranges per degree l
F32 = mybir.dt.float32
BF16 = mybir.dt.bfloat16
MAX = mybir.AluOpType.max
MULT = mybir.AluOpType.mult
EQ = mybir.AluOpType.is_equal

_cache = {}


def _build_bass():
    nc = bacc.Bacc("TRN2", target_bir_lowering=False, debug=False,
                   num_devices=N_CORES)
    x_in = nc.dram_tensor("x", [B, N, IRR, C], F32, kind="ExternalInput")
    out_t = nc.dram_tensor("out", [B, IRR, C], F32, kind="ExternalOutput")
    ident_d = nc.inline_tensor(np.eye(P, dtype=np.float32), name="ident")

    with tile.TileContext(nc) as tc, ExitStack() as ctx:
        # DRAM view: n = a*P + p  ->  [b, p, a, i, c]
        x_v = x_in.ap().rearrange("b (a p) i c -> b p a i c", p=P)
        out_v = out_t.ap().rearrange("b i c -> b (i c)")

        xp = ctx.enter_context(tc.tile_pool(name="xp", bufs=3))
        x2p = ctx.enter_context(tc.tile_pool(name="x2p", bufs=2))
        xbp = ctx.enter_context(tc.tile_pool(name="xbp", bufs=2))
        mtp = ctx.enter_context(tc.tile_pool(name="mtp", bufs=1))
        msp = ctx.enter_context(tc.tile_pool(name="msp", bufs=2))
        mvp = ctx.enter_context(tc.tile_pool(name="mvp", bufs=2))
        obp = ctx.enter_context(tc.tile_pool(name="obp", bufs=1))
        singles = ctx.enter_context(tc.tile_pool(name="singles", bufs=1))
        # PSUM banks: pnt 2 + pmp 2 + pout 2x2 = 8
        pnt = ctx.enter_context(tc.tile_pool(name="pnt", bufs=2, space="PSUM"))
        pmp = ctx.enter_context(tc.tile_pool(name="pmp", bufs=2, space="PSUM"))
        pout = ctx.enter_context(tc.tile_pool(name="pout", bufs=2, space="PSUM"))

        ones_b = singles.tile([P, 1], BF16)
        nc.vector.memset(ones_b, 1.0)
        ident = singles.tile([P, P], F32)
        nc.sync.dma_start(out=ident, in_=ident_d.ap())
        ident_b = singles.tile([P, P], BF16)
        nc.scalar.copy(out=ident_b, in_=ident)

        Xbs, POs = {}, {}

        def transpose_norms(l, X2):
            NT = pnt.tile([P, A * P], F32, tag="nt", name=f"NT{l}")
            s, e = BLOCKS[l]
            for a in range(A):
                sl = slice(a * P, (a + 1) * P)
                for i in range(s, e):
                    nc.tensor.matmul(NT[:, sl], X2[:, a, i, :], ident,
                                     is_transpose=True,
                                     start=(i == s), stop=(i == e - 1))
            return NT

        def mask_of(l, NT):
            maxv = mvp.tile([P, 1], F32, tag=f"maxv{l}", name=f"maxv{l}")
            nc.vector.tensor_reduce(
                out=maxv, in_=NT, axis=mybir.AxisListType.X, op=MAX)
            mt = mtp.tile([P, A * P], BF16, tag=f"mt{l}", name=f"mt{l}")
            nc.vector.tensor_scalar(
                out=mt, in0=NT, scalar1=maxv, scalar2=None, op0=EQ)
            mp = pmp.tile([P, A * P], BF16, tag="mp", name=f"mp{l}")
            for a in range(A):
                sl = slice(a * P, (a + 1) * P)
                nc.tensor.matmul(mp[:, sl], mt[:, sl], ident_b,
                                 is_transpose=True)
            return mp

        for it in range(B + 2):
            if it < B:
                b = it
                X = xp.tile([P, A, IRR, C], F32, tag="X", name="X")
                X2 = x2p.tile([P, A, IRR, C], F32, tag="X2", name="X2")
                Xb = xbp.tile([P, A, IRR, C], BF16, tag="Xb", name="Xb")
                Xbs[b] = Xb
                for a in range(A):
                    nc.sync.dma_start(out=X[:, a], in_=x_v[b][:, a])
                # per-chunk squares: each chunk's transposes start as soon
                # as that chunk is squared, shortening the chain to the
                # masks (worth the extra ACT instruction overhead)
                for a in range(A):
                    nc.scalar.activation(X2[:, a], X[:, a],
                                         mybir.ActivationFunctionType.Square)

                def wm_dve(l, mp):
                    s, e = BLOCKS[l]
                    mask = mp.rearrange("p (a c) -> p a c", a=A)
                    nc.vector.tensor_tensor(
                        Xb[:, :, s:e, :], X[:, :, s:e, :],
                        mask.unsqueeze(2).broadcast_to([P, A, e - s, C]),
                        MULT)

                def mask_sbuf(l, mp):
                    # GPSIMD cannot read PSUM: bounce the mask via ACT
                    ms = msp.tile([P, A, C], BF16, tag=f"ms{l}",
                                  name=f"ms{l}")
                    nc.scalar.copy(out=ms,
                                   in_=mp.rearrange("p (a c) -> p a c", a=A))
                    return ms

                def wm_gps(l, ms):
                    s, e = BLOCKS[l]
                    nc.gpsimd.tensor_tensor(
                        Xb[:, :, s:e, :], X[:, :, s:e, :],
                        ms.unsqueeze(2).broadcast_to([P, A, e - s, C]),
                        MULT)

                def emit_msum(bm):
                    # masked sum over n for batch bm, one iteration late so
                    # the PE never stalls on bm's winner-select
                    Xf = Xbs[bm].rearrange("p a i c -> p a (i c)")
                    POs[bm] = []
                    for h in range(2):
                        po = pout.tile([1, 2, 512], F32, tag="po", name="po")
                        for kk in range(2):
                            k = 2 * h + kk
                            for a in range(A):
                                nc.tensor.matmul(
                                    po[:, kk, :], ones_b,
                                    Xf[:, a, k * 512:(k + 1) * 512],
                                    start=(a == 0), stop=(a == A - 1))
                        POs[bm].append(po)

                # l3 FIRST: the GPSIMD winner-mult (7us, the longest X
                # reader) gets its mask as early as possible. Each l's
                # max/EQ runs under the next l's norm transposes, and DVE's
                # EQs all flow BEFORE its winner-mults (a winner-mult in
                # between would stall the PE's next mask transpose). The
                # previous batch's masked sum slots into the PE program
                # mid-iteration, after mask-l2's transpose.
                NT3 = transpose_norms(3, X2)
                NT2 = transpose_norms(2, X2)
                MP3 = mask_of(3, NT3)
                NT1 = transpose_norms(1, X2)
                wm_gps(3, mask_sbuf(3, MP3))
                MP2 = mask_of(2, NT2)
                NT0 = transpose_norms(0, X2)
                MP1 = mask_of(1, NT1)
                wm_dve(2, MP2)
                MP0 = mask_of(0, NT0)
                wm_dve(1, MP1)
                wm_dve(0, MP0)

            if it >= 2:
                # out copy + store for batch it-2, per k-chunk in completion
                # order so the out DMA fires as soon as possible
                bo = it - 2
                ob = obp.tile([1, IRR * C], F32, tag="ob", name="ob")
                for h in range(2):
                    nc.vector.tensor_copy(ob[:, h * 1024:(h + 1) * 1024],
                                          POs[bo][h].rearrange("m k f -> m (k f)"))
                # copies on DVE, DMA-issue on GPSIMD: on ACT either would
                # block the next batch's squares behind the masked-sum
                # wait, and on the sync queue the DMA would head-of-line
                # block input DMAs
                nc.gpsimd.dma_start(out=out_v[bo].unsqueeze(0), in_=ob)

            if 1 <= it <= B:
                emit_msum(it - 1)

    nc.compile()
    return nc


def kernel(x: np.ndarray, i2l: np.ndarray | None = None) -> np.ndarray:
    x = np.ascontiguousarray(np.asarray(x), dtype=np.float32)
    assert x.shape == (B_FULL, N, IRR, C), x.shape

    if "nc" not in _cache:
        _cache["nc"] = _build_bass()
    nc = _cache["nc"]

    from concourse.bass_utils import run_bass_kernel_spmd

    in_maps = [{"x": x[i * B:(i + 1) * B]} for i in range(N_CORES)]
    res = run_bass_kernel_spmd(nc, in_maps, list(range(N_CORES)))
    out = np.concatenate([res.results[i]["out"] for i in range(N_CORES)], axis=0)
    return out


if __name__ == "__main__":
    xs = np.random.randn(B_FULL, N, IRR, C).astype(np.float32)
    o = kernel(xs)
    print("out", o.shape, o.dtype)


# revision 32
# speedup vs baseline: 1.0522x; 1.0182x over previous
"""CoefficientMaxPool Trainium2 kernel (8-core data-parallel).

Problem: x [32, 512, 16, 128] f32.  Irreps group into degree blocks
l=0:[0,1), l=1:[1,4), l=2:[4,9), l=3:[9,16).  Per (batch, l, channel):
find the neighbor n* maximizing the degree-block squared norm, output
that neighbor's block components -> out [32, 16, 128].

Per core (4 batches), per batch:
  - DMA x[b] as [p=128, a=4, i=16, c=128] (n = a*128 + p), per-a chunks
  - ACT: x2 = x*x
  - PE transpose-accumulate x2 i-planes -> NT_l [c, n] in PSUM (fp32,
    exact argmax), processed per degree l (l3 first) so only 2 NT
    banks are live and the longest downstream chain starts earliest
  - DVE: maxv_l[c] = reduce-max over n; maskT_l[c,n] = (NT_l == maxv_l)
    emitted as bf16 (0/1 exact, so no precision risk in the select)
  - PE: transpose bf16 masks back -> MP_l [n, (a c)] PSUM (1 cyc/row)
  - winner-select Xb = X * MP[l(i)] -> bf16: l3 on GPSIMD (via an
    ACT-bounced SBUF mask copy; GPSIMD cannot read PSUM), l2+l1+l0 on
    DVE reading PSUM masks directly.  DVE's EQs all flow before its
    winner-mults so the PE's mask transposes are never starved.
  - PE: ones^T @ Xb (bf16: 1 cyc/row) accumulated over a -> out halves
    [1, 2, 512] fp32 in PSUM; ACT copy -> SBUF; ACT-issued DMA out.

Scheduling notes (each worth real microseconds on hardware):
  - The masked sum for batch b is emitted at the END of iteration b+1's
    PE program, so the PE fills batch b's winner-select wait with batch
    b+1's transposes.
  - The PSUM->SBUF out copy runs two iterations late: by then its PSUM
    source is guaranteed ready, so it never blocks ACT's squares.
  - Out DMAs are issued from the ACT queue: on the sync queue they
    would head-of-line block the next batch's input DMA.
  - X tiles are triple-buffered: the input DMA for batch b+3 only
    waits on batch b's last winner-mult, keeping the input stream at
    full HBM bandwidth (the single input queue is the ~47us floor).
PSUM banks: 2 (NT ring) + 2 (mask ring) + 4 (out halves ring) = 8.
"""

import os
import sys

import numpy as np

for _p in ("/opt/trn_rl_repo", "/opt/pypackages"):
    if _p not in sys.path:
        sys.path.append(_p)

from contextlib import ExitStack

import concourse.bacc as bacc
import concourse.bass as bass
import concourse.tile as tile
from concourse import mybir

N_CORES = 8
B_FULL, N, IRR, C = 32, 512, 16, 128
B = B_FULL // N_CORES  # 4 batches per core
P = 128                # partitions (n within chunk)
A = N // P             # 4 neighbor chunks
BLOCKS = [(0, 1), (1, 4), (4, 9), (9, 16)]  # irrep ranges per degree l
F32 = mybir.dt.float32
BF16 = mybir.dt.bfloat16
MAX = mybir.AluOpType.max
MULT = mybir.AluOpType.mult
EQ = mybir.AluOpType.is_equal

_cache = {}


def _build_bass():
    nc = bacc.Bacc("TRN2", target_bir_lowering=False, debug=False,
                   num_devices=N_CORES)
    x_in = nc.dram_tensor("x", [B, N, IRR, C], F32, kind="ExternalInput")
    out_t = nc.dram_tensor("out", [B, IRR, C], F32, kind="ExternalOutput")
    ident_d = nc.inline_tensor(np.eye(P, dtype=np.float32), name="ident")

    with tile.TileContext(nc) as tc, ExitStack() as ctx:
        # DRAM view: n = a*P + p  ->  [b, p, a, i, c]
        x_v = x_in.ap().rearrange("b (a p) i c -> b p a i c", p=P)
        out_v = out_t.ap().rearrange("b i c -> b (i c)")

        xp = ctx.enter_context(tc.tile_pool(name="xp", bufs=3))
        x2p = ctx.enter_context(tc.tile_pool(name="x2p", bufs=2))
        xbp = ctx.enter_context(tc.tile_pool(name="xbp", bufs=2))
        mtp = ctx.enter_context(tc.tile_pool(name="mtp", bufs=1))
        msp = ctx.enter_context(tc.tile_pool(name="msp", bufs=2))
        mvp = ctx.enter_context(tc.tile_pool(name="mvp", bufs=2))
        obp = ctx.enter_context(tc.tile_pool(name="obp", bufs=1))
        singles = ctx.enter_context(tc.tile_pool(name="singles", bufs=1))
        # PSUM banks: pnt 2 + pmp 2 + pout 2x2 = 8
        pnt = ctx.enter_context(tc.tile_pool(name="pnt", bufs=2, space="PSUM"))
        pmp = ctx.enter_context(tc.tile_pool(name="pmp", bufs=2, space="PSUM"))
        pout = ctx.enter_context(tc.tile_pool(name="pout", bufs=2, space="PSUM"))

        ones_b = singles.tile([P, 1], BF16)
        nc.vector.memset(ones_b, 1.0)
        ident = singles.tile([P, P], F32)
        nc.sync.dma_start(out=ident, in_=ident_d.ap())
        ident_b = singles.tile([P, P], BF16)
        nc.scalar.copy(out=ident_b, in_=ident)

        Xbs, POs = {}, {}

        def transpose_norms(l, X2):
            NT = pnt.tile([P, A * P], F32, tag="nt", name=f"NT{l}")
            s, e = BLOCKS[l]
            for a in range(A):
                sl = slice(a * P, (a + 1) * P)
                for i in range(s, e):
                    nc.tensor.matmul(NT[:, sl], X2[:, a, i, :], ident,
                                     is_transpose=True,
                                     start=(i == s), stop=(i == e - 1))
            return NT

        def mask_of(l, NT):
            maxv = mvp.tile([P, 1], F32, tag=f"maxv{l}", name=f"maxv{l}")
            nc.vector.tensor_reduce(
                out=maxv, in_=NT, axis=mybir.AxisListType.X, op=MAX)
            mt = mtp.tile([P, A * P], BF16, tag=f"mt{l}", name=f"mt{l}")
            nc.vector.tensor_scalar(
                out=mt, in0=NT, scalar1=maxv, scalar2=None, op0=EQ)
            mp = pmp.tile([P, A * P], BF16, tag="mp", name=f"mp{l}")
            for a in range(A):
                sl = slice(a * P, (a + 1) * P)
                nc.tensor.matmul(mp[:, sl], mt[:, sl], ident_b,
                                 is_transpose=True)
            return mp

        for it in range(B + 2):
            if it < B:
                b = it
                X = xp.tile([P, A, IRR, C], F32, tag="X", name="X")
                X2 = x2p.tile([P, A, IRR, C], F32, tag="X2", name="X2")
                Xb = xbp.tile([P, A, IRR, C], BF16, tag="Xb", name="Xb")
                Xbs[b] = Xb
                for a in range(A):
                    nc.sync.dma_start(out=X[:, a], in_=x_v[b][:, a])
                # per-chunk squares: each chunk's transposes start as soon
                # as that chunk is squared, shortening the chain to the
                # masks (worth the extra ACT instruction overhead)
                for a in range(A):
                    nc.scalar.activation(X2[:, a], X[:, a],
                                         mybir.ActivationFunctionType.Square)

                def wm_dve(l, mp):
                    s, e = BLOCKS[l]
                    mask = mp.rearrange("p (a c) -> p a c", a=A)
                    nc.vector.tensor_tensor(
                        Xb[:, :, s:e, :], X[:, :, s:e, :],
                        mask.unsqueeze(2).broadcast_to([P, A, e - s, C]),
                        MULT)

                def mask_sbuf(l, mp):
                    # GPSIMD cannot read PSUM: bounce the mask via ACT
                    ms = msp.tile([P, A, C], BF16, tag=f"ms{l}",
                                  name=f"ms{l}")
                    nc.scalar.copy(out=ms,
                                   in_=mp.rearrange("p (a c) -> p a c", a=A))
                    return ms

                def wm_gps(l, ms):
                    s, e = BLOCKS[l]
                    nc.gpsimd.tensor_tensor(
                        Xb[:, :, s:e, :], X[:, :, s:e, :],
                        ms.unsqueeze(2).broadcast_to([P, A, e - s, C]),
                        MULT)

                def emit_msum(bm):
                    # masked sum over n for batch bm, one iteration late so
                    # the PE never stalls on bm's winner-select
                    Xf = Xbs[bm].rearrange("p a i c -> p a (i c)")
                    POs[bm] = []
                    for h in range(2):
                        po = pout.tile([1, 2, 512], F32, tag="po", name="po")
                        for kk in range(2):
                            k = 2 * h + kk
                            for a in range(A):
                                nc.tensor.matmul(
                                    po[:, kk, :], ones_b,
                                    Xf[:, a, k * 512:(k + 1) * 512],
                                    start=(a == 0), stop=(a == A - 1))
                        POs[bm].append(po)

                # l3 FIRST: the GPSIMD winner-mult (7us, the longest X
                # reader) gets its mask as early as possible. Each l's
                # max/EQ runs under the next l's norm transposes, and DVE's
                # EQs all flow BEFORE its winner-mults (a winner-mult in
                # between would stall the PE's next mask transpose). The
                # previous batch's masked sum slots into the PE program
                # mid-iteration, after mask-l2's transpose.
                NT3 = transpose_norms(3, X2)
                NT2 = transpose_norms(2, X2)
                MP3 = mask_of(3, NT3)
                NT1 = transpose_norms(1, X2)
                wm_gps(3, mask_sbuf(3, MP3))
                MP2 = mask_of(2, NT2)
                NT0 = transpose_norms(0, X2)
                MP1 = mask_of(1, NT1)
                wm_dve(2, MP2)
                MP0 = mask_of(0, NT0)
                wm_dve(1, MP1)
                wm_dve(0, MP0)

            if it >= 2:
                # out copy + store for batch it-2, per k-chunk in completion
                # order so the out DMA fires as soon as possible
                bo = it - 2
                ob = obp.tile([1, IRR * C], F32, tag="ob", name="ob")
                for h in range(2):
                    nc.scalar.copy(out=ob[:, h * 1024:(h + 1) * 1024],
                                   in_=POs[bo][h].rearrange("m k f -> m (k f)"))
                # copies on DVE, DMA-issue on GPSIMD: on ACT either would
                # block the next batch's squares behind the masked-sum
                # wait, and on the sync queue the DMA would head-of-line
                # block input DMAs
                nc.gpsimd.dma_start(out=out_v[bo].unsqueeze(0), in_=ob)

            if 1 <= it <= B:
                emit_msum(it - 1)

    nc.compile()
    return nc


def kernel(x: np.ndarray, i2l: np.ndarray | None = None) -> np.ndarray:
    x = np.ascontiguousarray(np.asarray(x), dtype=np.float32)
    assert x.shape == (B_FULL, N, IRR, C), x.shape

    if "nc" not in _cache:
        _cache["nc"] = _build_bass()
    nc = _cache["nc"]

    from concourse.bass_utils import run_bass_kernel_spmd

    in_maps = [{"x": x[i * B:(i + 1) * B]} for i in range(N_CORES)]
    res = run_bass_kernel_spmd(nc, in_maps, list(range(N_CORES)))
    out = np.concatenate([res.results[i]["out"] for i in range(N_CORES)], axis=0)
    return out


if __name__ == "__main__":
    xs = np.random.randn(B_FULL, N, IRR, C).astype(np.float32)
    o = kernel(xs)
    print("out", o.shape, o.dtype)
